# revision 20
# baseline (speedup 1.0000x reference)
"""LocalCorrelation (13x13 cost volume) Trainium2 kernel.

Full inputs z_t, z_t1: [8, 256, 128, 128] f32 -> out [8, 169, 128, 128] f32.
out[b, 13*di+dj, h, w] = sum_c z_t[b,c,h,w] * pad(z_t1)[b,c,h+di,w+dj] / 16

Sharding: data-parallel over batch, 1 batch element per NeuronCore (8 cores).

Per-core algorithm (SPMD, identical program):
  - Slab-staged input load (f32->bf16 SWDGE cast) interleaved with compute;
    1/sqrt(C) scale fused into the z_t reorder copy; z_t stationary tiles
    streamed per-slab (rolling pool).
  - Per 8x16 output-pixel block: TensorE "block gram" matmuls, stationary =
    z_t block [c,128 pixels], streaming = padded z_t1 20x28 window -> PSUM
    f32 (accumulated over 2 c-chunks of 128).
  - PSUM -> SBUF bf16 (xb, per-pixel 20x28 window).
  - Dense-band scratch write: one DMA per (stripe, dh) covers all w of that
    pixel row; slicing the window rows p in [dh, dh+13) is uniform within
    the 16-partition dh-group, so scratch holds ONLY the needed 13-row band:
    scr[w*23296 + h*364 + (p-dh)*28 + q]. 728B runs.
  - Band gather: per (h-half, wb) one DMA reads a fully CONTIGUOUS 728KB
    stream (the band layout is sequential in (h, band)) -- full HBM rate.
  - DVE de-shear: tap-row pick + horizontal diagonal q = (w mod 16) + dj
    via strided copy (+ f32 cast) into per-di output tiles.
  - Output write: one DMA per (h-half, di), 512B runs; h-half 0 writes
    overlap h-half 1 gathers.
"""

import numpy as np

C = 256
H = W = 128
KS = 13
KK = 169
RAD = 6
HP = WP = 140  # padded spatial
SA = 8  # block rows (stripe height)
SB = 16  # block cols
NWB = W // SB  # 8 w-blocks per stripe
NST = H // SA  # 16 stripes
WINP = SA + 2 * RAD  # 20 streamed rows per window
WINQ = SB + 2 * RAD  # 28 streamed cols per window
PQ = WINP * WINQ  # 560 elems per pixel in xb
BAND = KS * WINQ  # 364: band elems per pixel (window rows dh..dh+12)
SCR_DW = SA * BAND  # 2912: scratch stride per dw (w mod 16)
SCR_HM = W * BAND  # 46592: scratch stride per h row (h-major layout)
HH = H // 2  # 64 rows per h-half
NDWG = 4  # dw-groups of 4 for the gather
GW = 4 * SCR_DW  # 11648: gathered elems per (h, dw-group)

_cache = {}


def _build():
    import concourse.bass as bass
    import concourse.mybir as mybir
    import concourse.tile as tile
    from concourse import bacc

    f32 = mybir.dt.float32
    bf16 = mybir.dt.bfloat16

    nc = bacc.Bacc("TRN2", target_bir_lowering=False, debug=False)
    zt_d = nc.dram_tensor("z_t", [C, H, W], f32, kind="ExternalInput")
    z1_d = nc.dram_tensor("z_t1", [C, H, W], f32, kind="ExternalInput")
    out_d = nc.dram_tensor("out", [KK, H, W], f32, kind="ExternalOutput")

    def alt(i):
        return nc.sync if i % 2 == 0 else nc.scalar

    with tile.TileContext(nc) as tc:
        with tc.tile_pool(name="scrp", bufs=1, space="DRAM") as scrp:
            scr_t = [scrp.tile([HH, SCR_HM], bf16, tag=f"scr{i}", name=f"scr{i}")
                     for i in range(2)]

            # ================= stripe phase =================
            with (
                tc.tile_pool(name="persist", bufs=1) as pp,
                tc.tile_pool(name="zts", bufs=4) as ztsp,
                tc.tile_pool(name="ztb", bufs=4) as ztbp,
                tc.tile_pool(name="xbp", bufs=2) as xbp,
                tc.tile_pool(name="psp", bufs=4, space="PSUM") as psp,
            ):
                Z1P = [pp.tile([128, HP * WP], bf16, tag=f"z1p{k}", name=f"z1p{k}")
                       for k in range(2)]
                for k in range(2):
                    # zero only the 6-wide pad frame, not the whole tile
                    zv = Z1P[k].rearrange("c (h w) -> c h w", h=HP)
                    nc.vector.memset(zv[:, 0:RAD, :], 0.0)
                    nc.vector.memset(zv[:, HP - RAD:HP, :], 0.0)
                    nc.vector.memset(zv[:, RAD:HP - RAD, 0:RAD], 0.0)
                    nc.vector.memset(zv[:, RAD:HP - RAD, WP - RAD:WP], 0.0)

                zt_tiles = {}

                def emit_loads(s):
                    """Direct SWDGE cast-DMAs of 32-row slabs (z_t1 into
                    the padded Z1P interior, z_t contiguous)."""
                    for k in range(2):
                        dst = Z1P[k].rearrange("c (h w) -> c h w", h=HP)[
                            :, RAD + s * 32: RAD + (s + 1) * 32, RAD: RAD + W]
                        src = z1_d.ap()[k * 128:(k + 1) * 128, s * 32:(s + 1) * 32, :]
                        nc.gpsimd.dma_start(dst, src)
                    for k in range(2):
                        zts = ztsp.tile([128, 4 * 1024], bf16, tag=f"zts{k}",
                                        name=f"zts{k}")
                        for sl in range(4):
                            zt_tiles[(s * 4 + sl, k)] = zts
                        src = zt_d.ap()[k * 128:(k + 1) * 128, s * 32:(s + 1) * 32, :]
                        nc.gpsimd.dma_start(
                            zts.rearrange("c (h w) -> c h w", h=32), src)

                def emit_stripe(si):
                    hh, sil = divmod(si, 8)
                    h0 = si * SA
                    # block-major stationary for this stripe (GpSimd copy)
                    ztb = {}
                    for k in range(2):
                        t = ztbp.tile([128, SA * W], bf16, tag=f"ztb{k}",
                                      name=f"ztb{k}")
                        ztb[k] = t
                        srcv = zt_tiles[(si, k)].rearrange(
                            "c (h wb dw) -> c wb h dw", h=32, wb=NWB)[
                            :, :, (si % 4) * SA:(si % 4 + 1) * SA, :]
                        dstv = t.rearrange("c (wb dh dw) -> c wb dh dw",
                                           wb=NWB, dh=SA)
                        nc.gpsimd.tensor_copy(dstv, srcv)
                    xb = xbp.tile([128, NWB * PQ], bf16, tag="xb", name="xb")
                    for wb in range(NWB):
                        w0 = wb * SB
                        ps = [psp.tile([128, 10 * WINQ], f32,
                                       tag=f"ps{i}", name=f"ps{i}")
                              for i in range(2)]
                        for k in range(2):
                            lhsT = ztb[k][:, wb * 128:(wb + 1) * 128]
                            for half in range(2):
                                rhs = Z1P[k].rearrange("c (h w) -> c h w", h=HP)[
                                    :, h0 + 10 * half: h0 + 10 * (half + 1),
                                    w0:w0 + WINQ]
                                nc.tensor.matmul(ps[half][:, :], lhsT, rhs,
                                                 start=(k == 0), stop=(k == 1))
                        for half in range(2):
                            dst = xb[:, wb * PQ + half * 10 * WINQ:
                                     wb * PQ + (half + 1) * 10 * WINQ]
                            if wb % 2 == 0:
                                nc.scalar.mul(dst, ps[half][:, :], 1.0 / 16.0)
                            else:
                                nc.vector.tensor_scalar_mul(
                                    dst, ps[half][:, :], 1.0 / 16.0)

                    # dense-band scratch write: one DMA per dh pixel-row;
                    # the p in [dh, dh+13) slice is uniform per dh-group.
                    # h-major scratch: addr = h*46592 + dw*2912 + wb*364 + e,
                    # fully contiguous per partition -> 5.8KB descriptors.
                    for dh in range(SA):
                        xsl = xb[dh * SB:(dh + 1) * SB, :]
                        src = bass.AP(xsl.tensor, xsl.offset + dh * WINQ,
                                      [list(xsl.ap[0]), [PQ, NWB], [1, BAND]])
                        dst = bass.AP(scr_t[hh].tensor,
                                      (sil * SA + dh) * SCR_HM,
                                      [[SCR_DW, SB], [BAND, NWB], [1, BAND]])
                        alt(dh).dma_start(dst, src)

                for s in range(4):
                    emit_loads(s)
                for si in range(NST):
                    emit_stripe(si)

            # ================= tap phase =================
            with (
                tc.tile_pool(name="bnd", bufs=2) as bndp,
                tc.tile_pool(name="o5p", bufs=1) as o5p,
            ):
                o5 = [o5p.tile([H, KS * W], f32, tag=f"o5_{di}", name=f"o5_{di}")
                      for di in range(KS)]
                for hh in range(2):
                    for dp in range(NDWG // 2):
                        # one gather covers a dwg-pair across both partition
                        # halves -> all 16 DMA engines per instruction.
                        bt = bndp.tile([128, GW], bf16, tag="band", name="band")
                        src = bass.AP(scr_t[hh].tensor, (2 * dp) * GW,
                                      [[GW, 2], [SCR_HM, HH], [1, GW]])
                        alt(dp).dma_start(bt[:, :], src)
                        # de-shear: o5[di][hh*64+p, dj*128 + wb*16 + dwg*4+dwl]
                        #   = band[p, dwl*2912 + wb*364 + di*28 + dwg*4+dwl + dj]
                        for g2 in range(2):
                            dwg = 2 * dp + g2
                            band = bt[g2 * HH:(g2 + 1) * HH, :]
                            for di in range(KS):
                                diag = bass.AP(
                                    band.tensor,
                                    band.offset + di * WINQ + dwg * NDWG,
                                    [list(band.ap[0]), [SCR_DW + 1, 4],
                                     [BAND, NWB], [1, KS]])
                                o5sl = o5[di][hh * HH:(hh + 1) * HH, :]
                                dst = bass.AP(o5sl.tensor,
                                              o5sl.offset + dwg * NDWG,
                                              [list(o5sl.ap[0]), [1, 4],
                                               [SB, NWB], [W, KS]])
                                if (dwg + di) % 2 == 0:
                                    nc.vector.tensor_copy(dst, diag)
                                else:
                                    nc.scalar.copy(dst, diag)
                    # output writes for this h-half (overlap next half's
                    # gathers)
                    for di in range(KS):
                        dstw = bass.AP(out_d, di * KS * H * W + hh * HH * W,
                                       [[W, HH], [H * W, KS], [1, W]])
                        alt(di).dma_start(
                            dstw, o5[di][hh * HH:(hh + 1) * HH, :].rearrange(
                                "p (dj w) -> p dj w", dj=KS))

    nc.compile()
    return nc


def _get_nc():
    if "nc" not in _cache:
        _cache["nc"] = _build()
    return _cache["nc"]


def kernel(z_t: np.ndarray, z_t1: np.ndarray) -> np.ndarray:
    from concourse.bass_utils import run_bass_kernel_spmd

    nc = _get_nc()
    z_t = np.ascontiguousarray(z_t, dtype=np.float32)
    z_t1 = np.ascontiguousarray(z_t1, dtype=np.float32)
    B = z_t.shape[0]
    in_maps = [{"z_t": z_t[i], "z_t1": z_t1[i]} for i in range(B)]
    res = run_bass_kernel_spmd(nc, in_maps, core_ids=list(range(B)))
    return np.stack([res.results[i]["out"] for i in range(B)], axis=0)


# revision 21
# speedup vs baseline: 1.6278x; 1.6278x over previous
"""LocalCorrelation (13x13 cost volume) Trainium2 kernel.

Full inputs z_t, z_t1: [8, 256, 128, 128] f32 -> out [8, 169, 128, 128] f32.
out[b, 13*di+dj, h, w] = sum_c z_t[b,c,h,w] * pad(z_t1)[b,c,h+di,w+dj] / 16

Sharding: data-parallel over batch, 1 batch element per NeuronCore (8 cores).

Per-core algorithm (SPMD, identical program):
  - Slab-staged input load (f32->bf16 SWDGE cast) interleaved with compute;
    1/sqrt(C) scale fused into the z_t reorder copy; z_t stationary tiles
    streamed per-slab (rolling pool).
  - Per 8x16 output-pixel block: TensorE "block gram" matmuls, stationary =
    z_t block [c,128 pixels], streaming = padded z_t1 20x28 window -> PSUM
    f32 (accumulated over 2 c-chunks of 128).
  - PSUM -> SBUF bf16 (xb, per-pixel 20x28 window).
  - Dense-band scratch write: one DMA per (stripe, dh) covers all w of that
    pixel row; slicing the window rows p in [dh, dh+13) is uniform within
    the 16-partition dh-group, so scratch holds ONLY the needed 13-row band:
    scr[w*23296 + h*364 + (p-dh)*28 + q]. 728B runs.
  - Band gather: per (h-half, wb) one DMA reads a fully CONTIGUOUS 728KB
    stream (the band layout is sequential in (h, band)) -- full HBM rate.
  - DVE de-shear: tap-row pick + horizontal diagonal q = (w mod 16) + dj
    via strided copy (+ f32 cast) into per-di output tiles.
  - Output write: one DMA per (h-half, di), 512B runs; h-half 0 writes
    overlap h-half 1 gathers.
"""

import numpy as np

C = 256
H = W = 128
KS = 13
KK = 169
RAD = 6
HP = WP = 140  # padded spatial
SA = 8  # block rows (stripe height)
SB = 16  # block cols
NWB = W // SB  # 8 w-blocks per stripe
NST = H // SA  # 16 stripes
WINP = SA + 2 * RAD  # 20 streamed rows per window
WINQ = SB + 2 * RAD  # 28 streamed cols per window
PQ = WINP * WINQ  # 560 elems per pixel in xb
BAND = KS * WINQ  # 364: band elems per pixel (window rows dh..dh+12)
SCR_DW = SA * BAND  # 2912: scratch stride per dw (w mod 16)
SCR_HM = W * BAND  # 46592: scratch stride per h row (h-major layout)
HH = H // 2  # 64 rows per h-half
NDWG = 4  # dw-groups of 4 for the gather
GW = 4 * SCR_DW  # 11648: gathered elems per (h, dw-group)

_cache = {}


def _build():
    import concourse.bass as bass
    import concourse.mybir as mybir
    import concourse.tile as tile
    from concourse import bacc

    f32 = mybir.dt.float32
    bf16 = mybir.dt.bfloat16

    nc = bacc.Bacc("TRN2", target_bir_lowering=False, debug=False)
    zt_d = nc.dram_tensor("z_t", [C, H, W], f32, kind="ExternalInput")
    z1_d = nc.dram_tensor("z_t1", [C, H, W], f32, kind="ExternalInput")
    out_d = nc.dram_tensor("out", [KK, H, W], f32, kind="ExternalOutput")

    def alt(i):
        return nc.sync if i % 2 == 0 else nc.scalar

    with tile.TileContext(nc) as tc:
        with tc.tile_pool(name="scrp", bufs=1, space="DRAM") as scrp:
            scr_t = [scrp.tile([HH, SCR_HM], bf16, tag=f"scr{i}", name=f"scr{i}")
                     for i in range(2)]

            # ================= stripe phase =================
            with (
                tc.tile_pool(name="persist", bufs=1) as pp,
                tc.tile_pool(name="zts", bufs=4) as ztsp,
                tc.tile_pool(name="ztb", bufs=4) as ztbp,
                tc.tile_pool(name="xbp", bufs=2) as xbp,
                tc.tile_pool(name="psp", bufs=4, space="PSUM") as psp,
            ):
                # z_t1 stored WITHOUT horizontal pad (full-width rows, fast
                # contiguous loads); horizontal-wrap garbage is zeroed later
                # in the gathered bands. 6-elem slack at both ends keeps the
                # window APs in-bounds. Vertical pad rows stay real zeros.
                Z1P = [pp.tile([128, HP * W + 2 * RAD], bf16,
                               tag=f"z1p{k}", name=f"z1p{k}")
                       for k in range(2)]
                for k in range(2):
                    nc.vector.memset(Z1P[k][:, 0:RAD * W + RAD], 0.0)
                    nc.vector.memset(
                        Z1P[k][:, RAD + (HP - RAD) * W:HP * W + 2 * RAD], 0.0)

                zt_tiles = {}

                def emit_loads(s):
                    """Direct SWDGE cast-DMAs of 32-row slabs (z_t1 into
                    the padded Z1P interior, z_t contiguous)."""
                    for k in range(2):
                        dst = Z1P[k][:, RAD + (RAD + s * 32) * W:
                                     RAD + (RAD + (s + 1) * 32) * W]
                        src = z1_d.ap()[k * 128:(k + 1) * 128, s * 32:(s + 1) * 32, :]
                        nc.gpsimd.dma_start(dst, src)
                    for k in range(2):
                        zts = ztsp.tile([128, 4 * 1024], bf16, tag=f"zts{k}",
                                        name=f"zts{k}")
                        for sl in range(4):
                            zt_tiles[(s * 4 + sl, k)] = zts
                        src = zt_d.ap()[k * 128:(k + 1) * 128, s * 32:(s + 1) * 32, :]
                        nc.gpsimd.dma_start(
                            zts.rearrange("c (h w) -> c h w", h=32), src)

                def emit_stripe(si):
                    hh, sil = divmod(si, 8)
                    h0 = si * SA
                    # block-major stationary for this stripe (GpSimd copy)
                    ztb = {}
                    for k in range(2):
                        t = ztbp.tile([128, SA * W], bf16, tag=f"ztb{k}",
                                      name=f"ztb{k}")
                        ztb[k] = t
                        srcv = zt_tiles[(si, k)].rearrange(
                            "c (h wb dw) -> c wb h dw", h=32, wb=NWB)[
                            :, :, (si % 4) * SA:(si % 4 + 1) * SA, :]
                        dstv = t.rearrange("c (wb dh dw) -> c wb dh dw",
                                           wb=NWB, dh=SA)
                        nc.gpsimd.tensor_copy(dstv, srcv)
                    xb = xbp.tile([128, NWB * PQ], bf16, tag="xb", name="xb")
                    for wb in range(NWB):
                        w0 = wb * SB
                        ps = [psp.tile([128, 10 * WINQ], f32,
                                       tag=f"ps{i}", name=f"ps{i}")
                              for i in range(2)]
                        for k in range(2):
                            lhsT = ztb[k][:, wb * 128:(wb + 1) * 128]
                            for half in range(2):
                                roff = RAD + (h0 + 10 * half) * W + w0 - RAD
                                rhs = bass.AP(Z1P[k][:, :].tensor,
                                              Z1P[k][:, :].offset + roff,
                                              [list(Z1P[k][:, :].ap[0]),
                                               [W, 10], [1, WINQ]])
                                nc.tensor.matmul(ps[half][:, :], lhsT, rhs,
                                                 start=(k == 0), stop=(k == 1))
                        for half in range(2):
                            dst = xb[:, wb * PQ + half * 10 * WINQ:
                                     wb * PQ + (half + 1) * 10 * WINQ]
                            if wb % 2 == 0:
                                nc.scalar.mul(dst, ps[half][:, :], 1.0 / 16.0)
                            else:
                                nc.vector.tensor_scalar_mul(
                                    dst, ps[half][:, :], 1.0 / 16.0)

                    # dense-band scratch write: one DMA per dh pixel-row;
                    # the p in [dh, dh+13) slice is uniform per dh-group.
                    # h-major scratch: addr = h*46592 + dw*2912 + wb*364 + e,
                    # fully contiguous per partition -> 5.8KB descriptors.
                    for dh in range(SA):
                        xsl = xb[dh * SB:(dh + 1) * SB, :]
                        src = bass.AP(xsl.tensor, xsl.offset + dh * WINQ,
                                      [list(xsl.ap[0]), [PQ, NWB], [1, BAND]])
                        dst = bass.AP(scr_t[hh].tensor,
                                      (sil * SA + dh) * SCR_HM,
                                      [[SCR_DW, SB], [BAND, NWB], [1, BAND]])
                        alt(dh).dma_start(dst, src)

                for s in range(4):
                    emit_loads(s)
                for si in range(NST):
                    emit_stripe(si)

            # ================= tap phase =================
            with (
                tc.tile_pool(name="bnd", bufs=2) as bndp,
                tc.tile_pool(name="o5p", bufs=1) as o5p,
            ):
                o5 = [o5p.tile([H, KS * W], f32, tag=f"o5_{di}", name=f"o5_{di}")
                      for di in range(KS)]
                for hh in range(2):
                    for dwg in range(NDWG):
                        par = (hh * NDWG + dwg) % 2
                        bt = bndp.tile([128, GW], bf16, tag="band", name="band")
                        band = bt[par * HH:(par + 1) * HH, :]
                        src = bass.AP(scr_t[hh].tensor, dwg * GW,
                                      [[SCR_HM, HH], [1, GW]])
                        alt(dwg).dma_start(band, src)
                        # zero horizontal-wrap garbage: wb=0 cols q<6 and
                        # wb=7 cols q>=22 (z1 col w+q-6 out of [0,128))
                        zl = bass.AP(band.tensor, band.offset,
                                     [list(band.ap[0]), [SCR_DW, 4],
                                      [WINQ, KS], [1, RAD]])
                        zr = bass.AP(band.tensor,
                                     band.offset + 7 * BAND + WINQ - RAD,
                                     [list(band.ap[0]), [SCR_DW, 4],
                                      [WINQ, KS], [1, RAD]])
                        nc.vector.memset(zl, 0.0)
                        nc.scalar.memzero(zr)
                        # de-shear: o5[di][hh*64+p, dj*128 + wb*16 + dwg*4+dwl]
                        #   = band[p, dwl*2912 + wb*364 + di*28 + dwg*4+dwl + dj]
                        for di in range(KS):
                            diag = bass.AP(band.tensor,
                                           band.offset + di * WINQ + dwg * NDWG,
                                           [list(band.ap[0]), [SCR_DW + 1, 4],
                                            [BAND, NWB], [1, KS]])
                            o5sl = o5[di][hh * HH:(hh + 1) * HH, :]
                            dst = bass.AP(o5sl.tensor,
                                          o5sl.offset + dwg * NDWG,
                                          [list(o5sl.ap[0]), [1, 4],
                                           [SB, NWB], [W, KS]])
                            if (dwg + di) % 2 == 0:
                                nc.vector.tensor_copy(dst, diag)
                            else:
                                nc.scalar.copy(dst, diag)
                    # output writes for this h-half (overlap next half's
                    # gathers)
                    for di in range(KS):
                        dstw = bass.AP(out_d, di * KS * H * W + hh * HH * W,
                                       [[W, HH], [H * W, KS], [1, W]])
                        alt(di).dma_start(
                            dstw, o5[di][hh * HH:(hh + 1) * HH, :].rearrange(
                                "p (dj w) -> p dj w", dj=KS))

    nc.compile()
    return nc


def _get_nc():
    if "nc" not in _cache:
        _cache["nc"] = _build()
    return _cache["nc"]


def kernel(z_t: np.ndarray, z_t1: np.ndarray) -> np.ndarray:
    from concourse.bass_utils import run_bass_kernel_spmd

    nc = _get_nc()
    z_t = np.ascontiguousarray(z_t, dtype=np.float32)
    z_t1 = np.ascontiguousarray(z_t1, dtype=np.float32)
    B = z_t.shape[0]
    in_maps = [{"z_t": z_t[i], "z_t1": z_t1[i]} for i in range(B)]
    res = run_bass_kernel_spmd(nc, in_maps, core_ids=list(range(B)))
    return np.stack([res.results[i]["out"] for i in range(B)], axis=0)
